# revision 1
# baseline (speedup 1.0000x reference)
"""DiagonalLinear (Toeplitz linear layer) Trainium2 kernel.

y[b,s,o] = sum_i x[b,s,i] * W[o,i] + bias[o],  W[o,i] = vals[(i-o) mod (OUT+IN-1)]
x: [4, 2048, 4096] f32, diagonals: [8191] f32, bias: [4096] f32.

Strategy (8 NeuronCores, data parallel over the 8192 flattened rows):
 - Each core computes 1024 rows: y_c = x_c @ W.T + bias.
 - The Toeplitz weight is never materialized (64 MB); instead each core
   holds a compact SBUF expansion rv[p, u] = vals[(3968 + p - u) mod 8191]
   ([128 x 8064] ~ 4 MB) from which every [128k x 512n] weight tile is a
   plain free-dim slice: rhs(kc, nc) = rv[:, 3968 + 512*nc - 128*kc :+512].
 - x is fed pre-transposed per core (xT [4096, 1024], host-side layout
   prep during sharding) so lhsT chunks [128k, 128m] load contiguously.
 - PE: 32-deep PSUM accumulation per [128m x 512n] output tile, float32r
   (TF32-like single-pass fp32: ~1.5e-4 rel err, 4x faster than fp32).
 - DVE drains PSUM with a fused bias add, HWDGE DMA stores to y.
"""

import numpy as np

import bass_rust
import concourse.bass as bass
import concourse.mybir as mybir
import concourse.tile as tile
from concourse.bass_utils import run_bass_kernel_spmd

IN_F = 4096
OUT_F = 4096
NVALS = OUT_F + IN_F - 1  # 8191
B, S = 4, 2048
ROWS = B * S              # 8192
N_CORES = 8
M_PER_CORE = ROWS // N_CORES  # 1024

MT = 128                  # m tile (PSUM partition dim)
NT = 512                  # n tile (one PSUM bank of fp32)
KT = 128                  # k tile (PE contraction dim)
N_MC = M_PER_CORE // MT   # 8
N_NC = OUT_F // NT        # 8
N_KC = IN_F // KT         # 32
RV_F = (N_KC - 1) * KT + (N_NC - 1) * NT + NT  # 8064 free dim of rv
RV_C0 = (N_KC - 1) * KT   # 3968: rv column of (kc=last... ) base constant

_COMPILED = None


def _legalize_single_wait(nc):
    """This walrus build encodes at most one sync-wait per instruction;
    move extra waits onto carrier NoOps on the same engine."""
    for f in nc.m.functions:
        for blk in f.blocks:
            insts = blk.instructions
            new = []
            changed = False
            for inst in insts:
                si = inst.sync_info
                if si is not None and si.on_wait is not None and len(si.on_wait) > 1:
                    waits = list(si.on_wait)
                    for w in waits[:-1]:
                        nop = mybir.InstNoOp(name=f"I-waitsplit-{nc.next_id()}")
                        nop.engine = inst.engine
                        nop.sync_info = bass_rust.SyncInfo(on_wait=[w], on_update=[])
                        new.append(nop)
                    inst.sync_info = bass_rust.SyncInfo(
                        on_wait=[waits[-1]], on_update=si.on_update
                    )
                    changed = True
                new.append(inst)
            if changed:
                blk.instructions = new


def build_nc(dt_x=mybir.dt.float32r, dt_w=mybir.dt.float32r):
    f32 = mybir.dt.float32
    nc = bass.Bass()
    xT = nc.dram_tensor("xT", [IN_F, M_PER_CORE], dt_x, kind="ExternalInput")
    rv = nc.dram_tensor("rv", [128, RV_F], dt_w, kind="ExternalInput")
    bias_rep = nc.dram_tensor("bias_rep", [128, OUT_F], f32, kind="ExternalInput")
    y = nc.dram_tensor("y", [M_PER_CORE, OUT_F], f32, kind="ExternalOutput")

    xT_r = xT.rearrange("(kc p) m -> p kc m", p=128)  # [128, N_KC, M_PER_CORE]

    with tile.TileContext(nc) as tc:
        with (
            tc.tile_pool(name="const", bufs=1) as cpool,
            tc.tile_pool(name="xp", bufs=2) as xpool,
            tc.tile_pool(name="op", bufs=4) as opool,
            tc.tile_pool(name="pp", bufs=4, space="PSUM") as ppool,
        ):
            # Startup-critical path: the first matmul (kc=31, nc=0) needs only
            # x chunks kc>=24 and rv cols [0,512). Issue DMAs in consumption
            # order (kc descending / rv cols ascending) so PE starts ~10us in
            # instead of waiting for the full 6MB of constants.
            xt_first = xpool.tile([128, N_KC, MT], dt_x, tag="xt")
            rv_sb = cpool.tile([128, RV_F], dt_w)
            bias_sb = cpool.tile([128, OUT_F], f32)

            def rv_load(c0, c1):
                nc.sync.dma_start(out=rv_sb[:, c0:c1], in_=rv[:, c0:c1])

            def xt_load(k0, k1):
                nc.sync.dma_start(
                    out=xt_first[:, k0:k1, :], in_=xT_r[:, k0:k1, 0:MT]
                )

            xt_load(24, 32)
            rv_load(0, 1024)
            xt_load(16, 24)
            rv_load(1024, 2048)
            xt_load(8, 16)
            xt_load(0, 8)
            rv_load(2048, 3072)
            nc.sync.dma_start(out=bias_sb, in_=bias_rep[:, :])
            for c0 in range(3072, RV_F, 1024):
                rv_load(c0, min(c0 + 1024, RV_F))

            for mc in range(N_MC):
                m0 = mc * MT
                if mc == 0:
                    xt_sb = xt_first
                else:
                    xt_sb = xpool.tile([128, N_KC, MT], dt_x, tag="xt")
                    nc.sync.dma_start(out=xt_sb, in_=xT_r[:, :, m0 : m0 + MT])
                for ncol in range(N_NC):
                    n0 = ncol * NT
                    acc = ppool.tile([MT, NT], f32, tag="acc")
                    for kk, kc in enumerate(reversed(range(N_KC))):
                        c = RV_C0 + n0 - kc * KT
                        nc.tensor.matmul(
                            acc,
                            xt_sb[:, kc, :],
                            rv_sb[:, c : c + NT],
                            start=(kk == 0),
                            stop=(kk == N_KC - 1),
                        )
                    out_sb = opool.tile([MT, NT], f32, tag="out")
                    nc.vector.tensor_add(out_sb, acc, bias_sb[:, n0 : n0 + NT])
                    nc.sync.dma_start(
                        out=y[m0 : m0 + MT, n0 : n0 + NT], in_=out_sb
                    )
    _legalize_single_wait(nc)
    return nc


def _prep_shared(diagonals, bias):
    vals = np.concatenate([diagonals[OUT_F - 1 :], diagonals[: OUT_F - 1]])
    p = np.arange(128)[:, None]
    u = np.arange(RV_F)[None, :]
    rv = vals[(RV_C0 + p - u) % NVALS].astype(np.float32)
    rv = np.ascontiguousarray(rv)
    bias_rep = np.ascontiguousarray(
        np.broadcast_to(bias.astype(np.float32), (128, OUT_F))
    )
    return rv, bias_rep


def kernel(x, diagonals, bias):
    global _COMPILED
    if _COMPILED is None:
        _COMPILED = build_nc()
    nc = _COMPILED

    x = np.asarray(x, dtype=np.float32)
    diagonals = np.asarray(diagonals, dtype=np.float32)
    bias = np.asarray(bias, dtype=np.float32)

    rv, bias_rep = _prep_shared(diagonals, bias)
    x2t = np.ascontiguousarray(x.reshape(ROWS, IN_F).T)  # [IN_F, ROWS]

    in_maps = []
    for c in range(N_CORES):
        sl = np.ascontiguousarray(
            x2t[:, c * M_PER_CORE : (c + 1) * M_PER_CORE]
        )
        in_maps.append({"xT": sl, "rv": rv, "bias_rep": bias_rep})

    res = run_bass_kernel_spmd(nc, in_maps, core_ids=list(range(N_CORES)))
    y = np.concatenate([res.results[c]["y"] for c in range(N_CORES)], axis=0)
    return y.reshape(B, S, OUT_F)



# revision 2
# speedup vs baseline: 1.0438x; 1.0438x over previous
"""DiagonalLinear (Toeplitz linear layer) Trainium2 kernel.

y[b,s,o] = sum_i x[b,s,i] * W[o,i] + bias[o],  W[o,i] = vals[(i-o) mod (OUT+IN-1)]
x: [4, 2048, 4096] f32, diagonals: [8191] f32, bias: [4096] f32.

Strategy (8 NeuronCores, data parallel over the 8192 flattened rows):
 - Each core computes 1024 rows: y_c = x_c @ W.T + bias.
 - The Toeplitz weight is never materialized (64 MB); instead each core
   holds a compact SBUF expansion rv[p, u] = vals[(3968 + p - u) mod 8191]
   ([128 x 8064] ~ 4 MB) from which every [128k x 512n] weight tile is a
   plain free-dim slice: rhs(kc, nc) = rv[:, 3968 + 512*nc - 128*kc :+512].
 - x is fed pre-transposed per core (xT [4096, 1024], host-side layout
   prep during sharding) so lhsT chunks [128k, 128m] load contiguously.
 - PE: 32-deep PSUM accumulation per [128m x 512n] output tile, float32r
   (TF32-like single-pass fp32: ~1.5e-4 rel err, 4x faster than fp32).
 - DVE drains PSUM with a fused bias add, HWDGE DMA stores to y.
"""

import numpy as np

import bass_rust
import concourse.bass as bass
import concourse.mybir as mybir
import concourse.tile as tile
from concourse.bass_utils import run_bass_kernel_spmd

IN_F = 4096
OUT_F = 4096
NVALS = OUT_F + IN_F - 1  # 8191
B, S = 4, 2048
ROWS = B * S              # 8192
N_CORES = 8
M_PER_CORE = ROWS // N_CORES  # 1024

MT = 128                  # m tile (PSUM partition dim)
NT = 512                  # n tile (one PSUM bank of fp32)
KT = 128                  # k tile (PE contraction dim)
N_MC = M_PER_CORE // MT   # 8
N_NC = OUT_F // NT        # 8
N_KC = IN_F // KT         # 32
RV_F = (N_KC - 1) * KT + (N_NC - 1) * NT + NT  # 8064 free dim of rv
RV_C0 = (N_KC - 1) * KT   # 3968: rv column of (kc=last... ) base constant

_COMPILED = None


def _legalize_single_wait(nc):
    """This walrus build encodes at most one sync-wait per instruction;
    move extra waits onto carrier NoOps on the same engine."""
    for f in nc.m.functions:
        for blk in f.blocks:
            insts = blk.instructions
            new = []
            changed = False
            for inst in insts:
                si = inst.sync_info
                if si is not None and si.on_wait is not None and len(si.on_wait) > 1:
                    waits = list(si.on_wait)
                    for w in waits[:-1]:
                        nop = mybir.InstNoOp(name=f"I-waitsplit-{nc.next_id()}")
                        nop.engine = inst.engine
                        nop.sync_info = bass_rust.SyncInfo(on_wait=[w], on_update=[])
                        new.append(nop)
                    inst.sync_info = bass_rust.SyncInfo(
                        on_wait=[waits[-1]], on_update=si.on_update
                    )
                    changed = True
                new.append(inst)
            if changed:
                blk.instructions = new


def build_nc(dt_x=mybir.dt.float32r, dt_w=mybir.dt.float32r):
    f32 = mybir.dt.float32
    nc = bass.Bass()
    xT = nc.dram_tensor("xT", [IN_F, M_PER_CORE], dt_x, kind="ExternalInput")
    rv = nc.dram_tensor("rv", [128, RV_F], dt_w, kind="ExternalInput")
    bias_rep = nc.dram_tensor("bias_rep", [128, OUT_F], f32, kind="ExternalInput")
    y = nc.dram_tensor("y", [M_PER_CORE, OUT_F], f32, kind="ExternalOutput")

    xT_r = xT.rearrange("(kc p) m -> p kc m", p=128)  # [128, N_KC, M_PER_CORE]

    with tile.TileContext(nc) as tc:
        with (
            tc.tile_pool(name="const", bufs=1) as cpool,
            tc.tile_pool(name="xp", bufs=2) as xpool,
            tc.tile_pool(name="op", bufs=4) as opool,
            tc.tile_pool(name="pp", bufs=4, space="PSUM") as ppool,
        ):
            # Startup-critical path: the first matmul (kc=31, nc=0) needs only
            # x chunks kc>=24 and rv cols [0,512). Issue DMAs in consumption
            # order (kc descending / rv cols ascending) so PE starts ~10us in
            # instead of waiting for the full 6MB of constants.
            xt_first = xpool.tile([128, N_KC, MT], dt_x, tag="xt")
            rv_sb = cpool.tile([128, RV_F], dt_w)
            bias_sb = cpool.tile([128, OUT_F], f32)

            def rv_load(c0, c1):
                nc.sync.dma_start(out=rv_sb[:, c0:c1], in_=rv[:, c0:c1])

            def xt_load(k0, k1):
                nc.sync.dma_start(
                    out=xt_first[:, k0:k1, :], in_=xT_r[:, k0:k1, 0:MT]
                )

            xt_load(24, 32)
            rv_load(0, 1024)
            xt_load(16, 24)
            rv_load(1024, 2048)
            xt_load(8, 16)
            xt_load(0, 8)
            rv_load(2048, 3072)
            nc.sync.dma_start(out=bias_sb, in_=bias_rep[:, :])
            for c0 in range(3072, RV_F, 1024):
                rv_load(c0, min(c0 + 1024, RV_F))

            for mc in range(N_MC):
                m0 = mc * MT
                if mc == 0:
                    xt_sb = xt_first
                else:
                    xt_sb = xpool.tile([128, N_KC, MT], dt_x, tag="xt")
                    nc.sync.dma_start(out=xt_sb, in_=xT_r[:, :, m0 : m0 + MT])
                for ncol in range(N_NC):
                    n0 = ncol * NT
                    acc = ppool.tile([MT, NT], f32, tag="acc")
                    for kk, kc in enumerate(reversed(range(N_KC))):
                        c = RV_C0 + n0 - kc * KT
                        nc.tensor.matmul(
                            acc,
                            xt_sb[:, kc, :],
                            rv_sb[:, c : c + NT],
                            start=(kk == 0),
                            stop=(kk == N_KC - 1),
                        )
                    out_sb = opool.tile([MT, NT], f32, tag="out")
                    nc.vector.tensor_add(out_sb, acc, bias_sb[:, n0 : n0 + NT])
                    nc.sync.dma_start(
                        out=y[m0 : m0 + MT, n0 : n0 + NT], in_=out_sb
                    )
    _legalize_single_wait(nc)
    return nc


def _prep_shared(diagonals, bias):
    vals = np.concatenate([diagonals[OUT_F - 1 :], diagonals[: OUT_F - 1]])
    p = np.arange(128)[:, None]
    u = np.arange(RV_F)[None, :]
    rv = vals[(RV_C0 + p - u) % NVALS].astype(np.float32)
    rv = np.ascontiguousarray(rv)
    bias_rep = np.ascontiguousarray(
        np.broadcast_to(bias.astype(np.float32), (128, OUT_F))
    )
    return rv, bias_rep


def make_in_maps(x, diagonals, bias):
    x = np.asarray(x, dtype=np.float32)
    diagonals = np.asarray(diagonals, dtype=np.float32)
    bias = np.asarray(bias, dtype=np.float32)

    rv, bias_rep = _prep_shared(diagonals, bias)
    x2t = np.ascontiguousarray(x.reshape(ROWS, IN_F).T)  # [IN_F, ROWS]

    in_maps = []
    for c in range(N_CORES):
        sl = np.ascontiguousarray(
            x2t[:, c * M_PER_CORE : (c + 1) * M_PER_CORE]
        )
        in_maps.append({"xT": sl, "rv": rv, "bias_rep": bias_rep})
    return in_maps


def kernel(x, diagonals, bias):
    global _COMPILED
    if _COMPILED is None:
        _COMPILED = build_nc()
    nc = _COMPILED

    in_maps = make_in_maps(x, diagonals, bias)
    res = run_bass_kernel_spmd(nc, in_maps, core_ids=list(range(N_CORES)))
    y = np.concatenate([res.results[c]["y"] for c in range(N_CORES)], axis=0)
    return y.reshape(B, S, OUT_F)

